# revision 27
# baseline (speedup 1.0000x reference)
"""Trainium2 Bass kernel for:
    tanh( (x0*x1 + sin(x2)) * exp(-|x3|) + x4 / (x5*x5 + exp(x6)) - x7 )
over inputs (8388608, 8) f32, data-parallel over 8 NeuronCores.

Final design (sustained DMA ~310 GB/s/core makes this memory/DVE bound):
  - Host marshals inputs to var-major fp16 (rel-err gate is 2e-2; this
    kernel lands ~9e-4): xs[7, R] holds vars ordered [x0,x1,x3,x6,x4,
    x5,x7] so the two exp operands (-|x3|, x6) are adjacent -> ONE fused
    2F-wide ACT exp. x2[R] ships separately (fp16) for the sin pass.
    Device traffic: 18 MB in + 2 MB out per core vs 36+4 MB at fp32.
  - Contiguous per-var SBUF slices enable DVE 2x mode everywhere.
  - Two ACT table-set phases TOTAL: pass A computes sin for the whole
    shard into a resident 16 KB/partition buffer (silu set), pass B
    does exp/square/tanh (exp_and_others). 2 table switches, not 2/batch.
  - Range reduction without the slow custom wrap op: ACT Copy computes
    round(x2/2pi)+1536 exactly via fp16 quantization (ulp(1536)=1), a
    second Copy maps it to k*(-2pi), one DVE add wraps x2 into [-pi,pi].
  - NO Pool/gpsimd compute: Pool shares an SBUF port with DVE and they
    serialize under sustained load (measured) - Pool TT costs ~3x the
    same op on DVE in shared time.
  - Division via fp16 Newton reciprocal: int16 magic seed (0x7798 -
    bits) + one NR step, all 2x-mode DVE ops; tt = (d*rs-2)*rs = -1/d
    (max rel err 3.2e-3), sign folded into w -= qn.
  - Engine balance: ACT exp(2F)/square/tanh + sin pass; DVE everything
    else; software-pipelined emission (tile t tail after tile t+1 head).
"""

import numpy as np

import concourse.bass as bass
import concourse.bacc as bacc
import concourse.mybir as mybir
from concourse.tile import TileContext
from concourse.tile_rust import add_dep_helper
from concourse import bass_utils

N_ROWS = 8_388_608
N_VARS = 8
N_CORES = 8
ROWS_PER_CORE = N_ROWS // N_CORES  # 1_048_576
P = 128
F = 1024                              # default tile free-dim
SHARD_F = ROWS_PER_CORE // P          # 8192 elems per partition per core

F32 = mybir.dt.float32
F16 = mybir.dt.float16
U16 = mybir.dt.uint16
AF = mybir.ActivationFunctionType
OP = mybir.AluOpType

# xs row order: x3 and x6 adjacent for the fused exp
XS_VARS = [0, 1, 3, 6, 4, 5, 7]
SLOT = {v: i for i, v in enumerate(XS_VARS)}


def build_bass(loop_iters: int = 1, ablate: str = "none",
               staggered: bool = False, f_size: int = F,
               xt_bufs: int = 6, tmp_bufs: int = 4,
               rcp: str = "fp16nr", dup: str = "none") -> bass.Bass:
    import contextlib
    FS = f_size
    TILE_ROWS = P * FS
    N_TILES = ROWS_PER_CORE // TILE_ROWS
    nc = bacc.Bacc("TRN2", debug=False, num_devices=N_CORES)
    xs = nc.dram_tensor("xs", [7, ROWS_PER_CORE], F16, kind="ExternalInput").ap()
    x2 = nc.dram_tensor("x2", [ROWS_PER_CORE], F16, kind="ExternalInput").ap()
    y = nc.dram_tensor("y", [ROWS_PER_CORE], F16, kind="ExternalOutput").ap()

    with TileContext(nc) as tc:
        with (
            tc.tile_pool(name="sin", bufs=1) as sin_pool,
            tc.tile_pool(name="pa", bufs=3) as pa_pool,
            tc.tile_pool(name="inp", bufs=xt_bufs) as inp_pool,
            tc.tile_pool(name="tmp", bufs=tmp_bufs) as tmp_pool,
            (tc.For_i(0, loop_iters, 1, staggered_reset=staggered)
             if loop_iters > 1 else contextlib.nullcontext()),
        ):
            stile = sin_pool.tile([P, N_TILES * FS], F16, name="stile")

            # ---- Pass A: sin(wrap(x2)) for the whole shard (silu set) ----
            # One up-front 2MB DMA so the sins are never starved behind the
            # big pass-B loads; wrap+sin in 2 wide chunks (fewer ops).
            x2all = sin_pool.tile([P, N_TILES * FS], F16, name="x2all")
            nc.sync.dma_start(
                out=x2all.rearrange("p (t f) -> p t f", t=N_TILES),
                in_=x2.rearrange("(t p f) -> p t f", t=N_TILES, p=P))
            last_sin = None
            if ablate not in ("dma", "nosin"):
                # range-reduce x2 into [-pi, pi] without the (slow) custom
                # wrap op: t1 = fp16(x2/(2pi) + 1536) == round(x2/(2pi)) + 1536
                # exactly (fp16 ulp at 1536 is 1), kk = (t1-1536)*(-2pi),
                # x2w = x2 + kk.  Sin's spline is only accurate on [-pi,pi].
                TWO_PI = float(2 * np.pi)
                for t in range(N_TILES):
                    sl = slice(t * FS, (t + 1) * FS)
                    t1 = pa_pool.tile([P, FS], F16, name=f"t1{t}", tag="t1")
                    nc.scalar.activation(t1, x2all[:, sl], AF.Copy,
                                         bias=1536.0, scale=1.0 / TWO_PI)
                    # kk = (t1-1536)*(-2pi), computed on ACT in fp32
                    # internals: t1*(-2pi) + 1536*2pi
                    kk = pa_pool.tile([P, FS], F16, name=f"kk{t}", tag="kk")
                    nc.scalar.activation(kk, t1, AF.Copy,
                                         bias=float(1536 * 2 * np.pi),
                                         scale=-TWO_PI)
                    nc.vector.tensor_tensor(out=kk, in0=x2all[:, sl], in1=kk,
                                            op=OP.add)
                    si = nc.scalar.activation(stile[:, sl], kk, AF.Sin)
                    last_sin = si.ins

            # ---- Pass B: everything else (exp_and_others set) ----
            # Software-pipelined emission: tile t's tail (w*e, w-qn, w-x7,
            # tanh, out-DMA) is emitted AFTER tile t+1's head so no engine
            # FIFO has a tail op blocking the next tile's head op.
            def emit_head(t):
                r0, r1 = t * TILE_ROWS, (t + 1) * TILE_ROWS
                xt = inp_pool.tile([P, 7 * FS], F16, name=f"xt{t}", tag="xt")
                nc.sync.dma_start(
                    out=xt.rearrange("p (v f) -> p v f", v=7),
                    in_=xs[:, r0:r1].rearrange("v (p f) -> p v f", p=P))
                v = {k: xt[:, s * FS:(s + 1) * FS] for k, s in SLOT.items()}
                if ablate == "dma":
                    nc.sync.dma_start(
                        out=y[r0:r1].rearrange("(p f) -> p f", p=P),
                        in_=v[7])
                    return None

                # -|x3| in place via sign-bit OR (DVE, 16-bit)
                nc.vector.tensor_scalar(
                    out=v[3].bitcast(U16), in0=v[3].bitcast(U16),
                    scalar1=0x8000, scalar2=None, op0=OP.bitwise_or)

                # fused exp over the adjacent [x3'|x6] 2F slice
                e2 = tmp_pool.tile([P, 2 * FS], F16, name=f"e2{t}", tag="e2")
                i1 = nc.scalar.activation(e2, xt[:, 2 * FS:4 * FS], AF.Exp)
                e, e6 = e2[:, 0:FS], e2[:, FS:2 * FS]

                # Pool/gpsimd shares an SBUF port with DVE (they serialize
                # under sustained load) -> NO Pool ops; sq rides on ACT.
                sq = tmp_pool.tile([P, FS], F16, name=f"sq{t}", tag="sq")
                i2 = nc.scalar.activation(sq, v[5], AF.Square)

                if rcp == "fp16nr":
                    # fp16 reciprocal: int16 magic seed + one Newton step,
                    # all 2x-mode fp16 DVE ops. tt = (d*rs - 2)*rs = -1/d
                    # (max rel err 3.2e-3); the sign folds into w -= qn.
                    d = tmp_pool.tile([P, FS], F16, name=f"d{t}", tag="d")
                    nc.vector.tensor_tensor(out=d, in0=sq, in1=e6, op=OP.add)
                    rs = tmp_pool.tile([P, FS], F16, name=f"rs{t}", tag="rs")
                    nc.vector.tensor_scalar(
                        out=rs.bitcast(mybir.dt.int16),
                        in0=d.bitcast(mybir.dt.int16),
                        scalar1=-1, scalar2=0x7798,
                        op0=OP.mult, op1=OP.add)
                    tt = tmp_pool.tile([P, FS], F16, name=f"tt{t}", tag="tt")
                    nc.vector.tensor_tensor(out=tt, in0=d, in1=rs, op=OP.mult)
                    nc.vector.scalar_tensor_tensor(
                        out=tt, in0=tt, scalar=2.0, in1=rs,
                        op0=OP.subtract, op1=OP.mult)
                    qn = tmp_pool.tile([P, FS], F16, name=f"qn{t}", tag="qn")
                    nc.vector.tensor_tensor(out=qn, in0=v[4], in1=tt,
                                            op=OP.mult)
                else:
                    d32 = tmp_pool.tile([P, FS], F32, name=f"d32{t}", tag="d32")
                    nc.vector.tensor_tensor(out=d32, in0=sq, in1=e6, op=OP.add)
                    nc.vector.reciprocal_approx_fast(out=d32, in_=d32)
                    qn = tmp_pool.tile([P, FS], F16, name=f"qn{t}", tag="qn")
                    nc.vector.scalar_tensor_tensor(
                        out=qn, in0=v[4], scalar=-1.0, in1=d32,
                        op0=OP.mult, op1=OP.mult)

                if dup == "dve":
                    scr = tmp_pool.tile([P, FS], F16, name=f"scr{t}", tag="scr")
                    for _ in range(8):
                        nc.vector.tensor_tensor(out=scr, in0=v[0], in1=v[1],
                                                op=OP.mult)
                elif dup == "act":
                    scr = tmp_pool.tile([P, FS], F16, name=f"scr{t}", tag="scr")
                    sc2 = nc.scalar.activation(scr, v[0], AF.Square)
                    sc3 = nc.scalar.activation(scr, v[1], AF.Square)

                w = tmp_pool.tile([P, FS], F16, name=f"w{t}", tag="w")
                nc.vector.tensor_tensor(out=w, in0=v[0], in1=v[1], op=OP.mult)
                if ablate != "nosin":
                    nc.vector.tensor_tensor(
                        out=w, in0=w, in1=stile[:, t * FS:(t + 1) * FS],
                        op=OP.add)
                return (t, v, e, qn, w, i1, i2)

            def emit_tail(st):
                t, v, e, qn, w, i1, i2 = st
                r0, r1 = t * TILE_ROWS, (t + 1) * TILE_ROWS
                nc.vector.tensor_tensor(out=w, in0=w, in1=e, op=OP.mult)
                nc.vector.tensor_tensor(out=w, in0=w, in1=qn, op=OP.subtract)
                nc.vector.tensor_tensor(out=w, in0=w, in1=v[7], op=OP.subtract)
                i3 = nc.scalar.activation(w, w, AF.Tanh)
                o = w
                if last_sin is not None:
                    for bi in (i1, i3):
                        add_dep_helper(bi.ins, last_sin, False,
                                       "act-set phase order")
                nc.sync.dma_start(
                    out=y[r0:r1].rearrange("(p f) -> p f", p=P), in_=o)

            pending = None
            for t in range(N_TILES):
                st = emit_head(t)
                if pending is not None:
                    emit_tail(pending)
                pending = st
            if pending is not None:
                emit_tail(pending)
    nc.compile()
    return nc


_BUILT = None


def _get_built():
    global _BUILT
    if _BUILT is None:
        _BUILT = build_bass()
    return _BUILT


def make_in_maps(inputs: np.ndarray) -> list[dict]:
    x = np.asarray(inputs, dtype=np.float32)
    assert x.shape == (N_ROWS, N_VARS), x.shape
    xT = np.ascontiguousarray(x.T)           # [8, N]
    xs_all = xT[XS_VARS].astype(np.float16)  # [7, N] var-major
    x2_all = xT[2].astype(np.float16)        # [N]
    R = ROWS_PER_CORE
    return [
        {
            "xs": np.ascontiguousarray(xs_all[:, c * R:(c + 1) * R]),
            "x2": np.ascontiguousarray(x2_all[c * R:(c + 1) * R]),
        }
        for c in range(N_CORES)
    ]


def run_spmd(inputs: np.ndarray, **kwargs) -> tuple[np.ndarray, object]:
    """Shard, run on 8 cores, gather. Retries transient device wedges."""
    import time as _time
    in_maps = make_in_maps(inputs)
    nc = _get_built()
    last_exc = None
    for attempt in range(3):
        try:
            res = bass_utils.run_bass_kernel_spmd(
                nc, in_maps, core_ids=list(range(N_CORES)), **kwargs
            )
            break
        except Exception as exc:  # transient device wedge — retry
            last_exc = exc
            _time.sleep(10 * (attempt + 1))
    else:
        raise last_exc
    out = np.concatenate([r["y"].reshape(-1) for r in res.results], axis=0)
    return out.astype(np.float32), res


def kernel(inputs: np.ndarray) -> np.ndarray:
    out, _ = run_spmd(inputs)
    return out


# revision 28
# speedup vs baseline: 1.1457x; 1.1457x over previous
"""Trainium2 Bass kernel for:
    tanh( (x0*x1 + sin(x2)) * exp(-|x3|) + x4 / (x5*x5 + exp(x6)) - x7 )
over inputs (8388608, 8) f32, data-parallel over 8 NeuronCores.

Final design (sustained DMA ~310 GB/s/core makes this memory/DVE bound):
  - Host marshals inputs to var-major fp16 (rel-err gate is 2e-2; this
    kernel lands ~9e-4): xs[7, R] holds vars ordered [x0,x1,x3,x6,x4,
    x5,x7] so the two exp operands (-|x3|, x6) are adjacent -> ONE fused
    2F-wide ACT exp. x2[R] ships separately (fp16) for the sin pass.
    Device traffic: 18 MB in + 2 MB out per core vs 36+4 MB at fp32.
  - Contiguous per-var SBUF slices enable DVE 2x mode everywhere.
  - Two ACT table-set phases TOTAL: pass A computes sin for the whole
    shard into a resident 16 KB/partition buffer (silu set), pass B
    does exp/square/tanh (exp_and_others). 2 table switches, not 2/batch.
  - Range reduction without the slow custom wrap op: ACT Copy computes
    round(x2/2pi)+1536 exactly via fp16 quantization (ulp(1536)=1), a
    second Copy maps it to k*(-2pi), one DVE add wraps x2 into [-pi,pi].
  - NO Pool/gpsimd compute: Pool shares an SBUF port with DVE and they
    serialize under sustained load (measured) - Pool TT costs ~3x the
    same op on DVE in shared time.
  - Division via fp16 Newton reciprocal: int16 magic seed (0x7798 -
    bits) + one NR step, all 2x-mode DVE ops; tt = (d*rs-2)*rs = -1/d
    (max rel err 3.2e-3), sign folded into w -= qn.
  - Engine balance: ACT exp(2F)/square/tanh + sin pass; DVE everything
    else; software-pipelined emission (tile t tail after tile t+1 head).
"""

import numpy as np

import concourse.bass as bass
import concourse.bacc as bacc
import concourse.mybir as mybir
from concourse.tile import TileContext
from concourse.tile_rust import add_dep_helper
from concourse import bass_utils

N_ROWS = 8_388_608
N_VARS = 8
N_CORES = 8
ROWS_PER_CORE = N_ROWS // N_CORES  # 1_048_576
P = 128
F = 1024                              # default tile free-dim
SHARD_F = ROWS_PER_CORE // P          # 8192 elems per partition per core

F32 = mybir.dt.float32
F16 = mybir.dt.float16
U16 = mybir.dt.uint16
AF = mybir.ActivationFunctionType
OP = mybir.AluOpType

# xs row order: divide-chain vars (x3,x6,x5) first so they ride the
# first split-DMA; x3/x6 adjacent for the fused exp
XS_VARS = [3, 6, 5, 0, 1, 4, 7]
SLOT = {v: i for i, v in enumerate(XS_VARS)}


def build_bass(loop_iters: int = 1, ablate: str = "none",
               staggered: bool = False, f_size: int = F,
               xt_bufs: int = 6, tmp_bufs: int = 4,
               rcp: str = "fp16nr", dup: str = "none") -> bass.Bass:
    import contextlib
    FS = f_size
    TILE_ROWS = P * FS
    N_TILES = ROWS_PER_CORE // TILE_ROWS
    nc = bacc.Bacc("TRN2", debug=False, num_devices=N_CORES)
    xs = nc.dram_tensor("xs", [7, ROWS_PER_CORE], F16, kind="ExternalInput").ap()
    x2 = nc.dram_tensor("x2", [ROWS_PER_CORE], F16, kind="ExternalInput").ap()
    y = nc.dram_tensor("y", [ROWS_PER_CORE], F16, kind="ExternalOutput").ap()

    with TileContext(nc) as tc:
        with (
            tc.tile_pool(name="sin", bufs=1) as sin_pool,
            tc.tile_pool(name="pa", bufs=3) as pa_pool,
            tc.tile_pool(name="inp", bufs=xt_bufs) as inp_pool,
            tc.tile_pool(name="tmp", bufs=tmp_bufs) as tmp_pool,
            (tc.For_i(0, loop_iters, 1, staggered_reset=staggered)
             if loop_iters > 1 else contextlib.nullcontext()),
        ):
            stile = sin_pool.tile([P, N_TILES * FS], F16, name="stile")

            # ---- Pass A: sin(wrap(x2)) for the whole shard (silu set) ----
            # One up-front 2MB DMA so the sins are never starved behind the
            # big pass-B loads; wrap+sin in 2 wide chunks (fewer ops).
            x2all = sin_pool.tile([P, N_TILES * FS], F16, name="x2all")
            nc.sync.dma_start(
                out=x2all.rearrange("p (t f) -> p t f", t=N_TILES),
                in_=x2.rearrange("(t p f) -> p t f", t=N_TILES, p=P))
            last_sin = None
            if ablate not in ("dma", "nosin"):
                # range-reduce x2 into [-pi, pi] without the (slow) custom
                # wrap op: t1 = fp16(x2/(2pi) + 1536) == round(x2/(2pi)) + 1536
                # exactly (fp16 ulp at 1536 is 1), kk = (t1-1536)*(-2pi),
                # x2w = x2 + kk.  Sin's spline is only accurate on [-pi,pi].
                TWO_PI = float(2 * np.pi)
                # t1/kk on DVE 4x-mode tensor_scalar, NOT on ACT: the
                # table-set phasing serializes pass-A ACT with pass-B ACT,
                # so every ACT cycle spent here delays the whole pass B.
                for t in range(N_TILES):
                    sl = slice(t * FS, (t + 1) * FS)
                    t1 = pa_pool.tile([P, FS], F16, name=f"t1{t}", tag="t1")
                    nc.vector.tensor_scalar(
                        out=t1, in0=x2all[:, sl], scalar1=1.0 / TWO_PI,
                        scalar2=1536.0, op0=OP.mult, op1=OP.add)
                    kk = pa_pool.tile([P, FS], F16, name=f"kk{t}", tag="kk")
                    nc.vector.tensor_scalar(
                        out=kk, in0=t1, scalar1=1536.0, scalar2=-TWO_PI,
                        op0=OP.subtract, op1=OP.mult)
                    nc.vector.tensor_tensor(out=kk, in0=x2all[:, sl], in1=kk,
                                            op=OP.add)
                    si = nc.scalar.activation(stile[:, sl], kk, AF.Sin)
                    last_sin = si.ins

            # ---- Pass B: everything else (exp_and_others set) ----
            # Software-pipelined emission: tile t's tail (w*e, w-qn, w-x7,
            # tanh, out-DMA) is emitted AFTER tile t+1's head so no engine
            # FIFO has a tail op blocking the next tile's head op.
            def emit_head(t):
                r0, r1 = t * TILE_ROWS, (t + 1) * TILE_ROWS
                xt = inp_pool.tile([P, 7 * FS], F16, name=f"xt{t}", tag="xt")
                xt3 = xt.rearrange("p (v f) -> p v f", v=7)
                xsr = xs[:, r0:r1].rearrange("v (p f) -> p v f", p=P)
                # split load: divide-chain vars (x3,x6,x5) first so the
                # e2/d chain starts before the w-chain vars arrive
                nc.sync.dma_start(out=xt3[:, 0:3], in_=xsr[:, 0:3])
                nc.sync.dma_start(out=xt3[:, 3:7], in_=xsr[:, 3:7])
                v = {k: xt[:, s * FS:(s + 1) * FS] for k, s in SLOT.items()}
                if ablate == "dma":
                    nc.sync.dma_start(
                        out=y[r0:r1].rearrange("(p f) -> p f", p=P),
                        in_=v[7])
                    return None

                # -|x3| in place via sign-bit OR (DVE, 16-bit)
                nc.vector.tensor_scalar(
                    out=v[3].bitcast(U16), in0=v[3].bitcast(U16),
                    scalar1=0x8000, scalar2=None, op0=OP.bitwise_or)

                # fused exp over the adjacent [x3'|x6] 2F slice
                e2 = tmp_pool.tile([P, 2 * FS], F16, name=f"e2{t}", tag="e2")
                i1 = nc.scalar.activation(e2, xt[:, 0:2 * FS], AF.Exp)
                e, e6 = e2[:, 0:FS], e2[:, FS:2 * FS]

                # Pool/gpsimd shares an SBUF port with DVE (they serialize
                # under sustained load) -> NO Pool ops; sq rides on ACT.
                sq = tmp_pool.tile([P, FS], F16, name=f"sq{t}", tag="sq")
                i2 = nc.scalar.activation(sq, v[5], AF.Square)

                if rcp == "fp16nr":
                    # fp16 reciprocal: int16 magic seed + one Newton step,
                    # all 2x-mode fp16 DVE ops. tt = (d*rs - 2)*rs = -1/d
                    # (max rel err 3.2e-3); the sign folds into w -= qn.
                    d = tmp_pool.tile([P, FS], F16, name=f"d{t}", tag="d")
                    nc.vector.tensor_tensor(out=d, in0=sq, in1=e6, op=OP.add)
                    rs = tmp_pool.tile([P, FS], F16, name=f"rs{t}", tag="rs")
                    nc.vector.tensor_scalar(
                        out=rs.bitcast(mybir.dt.int16),
                        in0=d.bitcast(mybir.dt.int16),
                        scalar1=-1, scalar2=0x7798,
                        op0=OP.mult, op1=OP.add)
                    tt = tmp_pool.tile([P, FS], F16, name=f"tt{t}", tag="tt")
                    nc.vector.tensor_tensor(out=tt, in0=d, in1=rs, op=OP.mult)
                    nc.vector.scalar_tensor_tensor(
                        out=tt, in0=tt, scalar=2.0, in1=rs,
                        op0=OP.subtract, op1=OP.mult)
                    qn = tmp_pool.tile([P, FS], F16, name=f"qn{t}", tag="qn")
                    nc.vector.tensor_tensor(out=qn, in0=v[4], in1=tt,
                                            op=OP.mult)
                else:
                    d32 = tmp_pool.tile([P, FS], F32, name=f"d32{t}", tag="d32")
                    nc.vector.tensor_tensor(out=d32, in0=sq, in1=e6, op=OP.add)
                    nc.vector.reciprocal_approx_fast(out=d32, in_=d32)
                    qn = tmp_pool.tile([P, FS], F16, name=f"qn{t}", tag="qn")
                    nc.vector.scalar_tensor_tensor(
                        out=qn, in0=v[4], scalar=-1.0, in1=d32,
                        op0=OP.mult, op1=OP.mult)

                if dup == "dve":
                    scr = tmp_pool.tile([P, FS], F16, name=f"scr{t}", tag="scr")
                    for _ in range(8):
                        nc.vector.tensor_tensor(out=scr, in0=v[0], in1=v[1],
                                                op=OP.mult)
                elif dup == "act":
                    scr = tmp_pool.tile([P, FS], F16, name=f"scr{t}", tag="scr")
                    sc2 = nc.scalar.activation(scr, v[0], AF.Square)
                    sc3 = nc.scalar.activation(scr, v[1], AF.Square)

                w = tmp_pool.tile([P, FS], F16, name=f"w{t}", tag="w")
                nc.vector.tensor_tensor(out=w, in0=v[0], in1=v[1], op=OP.mult)
                if ablate != "nosin":
                    nc.vector.tensor_tensor(
                        out=w, in0=w, in1=stile[:, t * FS:(t + 1) * FS],
                        op=OP.add)
                return (t, v, e, qn, w, i1, i2)

            def emit_tail(st):
                t, v, e, qn, w, i1, i2 = st
                r0, r1 = t * TILE_ROWS, (t + 1) * TILE_ROWS
                nc.vector.tensor_tensor(out=w, in0=w, in1=e, op=OP.mult)
                nc.vector.tensor_tensor(out=w, in0=w, in1=qn, op=OP.subtract)
                nc.vector.tensor_tensor(out=w, in0=w, in1=v[7], op=OP.subtract)
                i3 = nc.scalar.activation(w, w, AF.Tanh)
                o = w
                if last_sin is not None:
                    for bi in (i1, i3):
                        add_dep_helper(bi.ins, last_sin, False,
                                       "act-set phase order")
                nc.sync.dma_start(
                    out=y[r0:r1].rearrange("(p f) -> p f", p=P), in_=o)

            pending = None
            for t in range(N_TILES):
                st = emit_head(t)
                if pending is not None:
                    emit_tail(pending)
                pending = st
            if pending is not None:
                emit_tail(pending)
    nc.compile()
    return nc


_BUILT = None


def _get_built():
    global _BUILT
    if _BUILT is None:
        _BUILT = build_bass()
    return _BUILT


def make_in_maps(inputs: np.ndarray) -> list[dict]:
    x = np.asarray(inputs, dtype=np.float32)
    assert x.shape == (N_ROWS, N_VARS), x.shape
    xT = np.ascontiguousarray(x.T)           # [8, N]
    xs_all = xT[XS_VARS].astype(np.float16)  # [7, N] var-major
    x2_all = xT[2].astype(np.float16)        # [N]
    R = ROWS_PER_CORE
    return [
        {
            "xs": np.ascontiguousarray(xs_all[:, c * R:(c + 1) * R]),
            "x2": np.ascontiguousarray(x2_all[c * R:(c + 1) * R]),
        }
        for c in range(N_CORES)
    ]


def run_spmd(inputs: np.ndarray, **kwargs) -> tuple[np.ndarray, object]:
    """Shard, run on 8 cores, gather. Retries transient device wedges."""
    import time as _time
    in_maps = make_in_maps(inputs)
    nc = _get_built()
    last_exc = None
    for attempt in range(3):
        try:
            res = bass_utils.run_bass_kernel_spmd(
                nc, in_maps, core_ids=list(range(N_CORES)), **kwargs
            )
            break
        except Exception as exc:  # transient device wedge — retry
            last_exc = exc
            _time.sleep(10 * (attempt + 1))
    else:
        raise last_exc
    out = np.concatenate([r["y"].reshape(-1) for r in res.results], axis=0)
    return out.astype(np.float32), res


def kernel(inputs: np.ndarray) -> np.ndarray:
    out, _ = run_spmd(inputs)
    return out
